# revision 13
# baseline (speedup 1.0000x reference)
"""Trainium2 Bass kernel for AttnPainterOil-style top-K stroke compositing.

Problem semantics (per pixel, fully independent):
  draw[n] = (n+1) * (alpha[n] > 0.1); top-K=10 of draw over N=256 strokes
  (descending) == the 10 highest-index strokes with alpha > 0.1 (for the
  target input distribution every pixel has >= 10 passing strokes, checked
  on the host below).  Gather alpha/color at those indices and composite
  back-to-front over a white canvas.

Streaming formulation used on device (front-to-back, strokes in descending
index order): maintain per-pixel transmittance T (init 1), accepted count k
(init 0) and color accumulator C (init 0).  For each stroke:
  ae = a * 1{a > 0.1} * 1{k < 10}
  k += 1{ae > 0}
  ta = ae * T ;  T -= ta ;  C += ta * c
Final canvas = C + T (white background).

Only the top D=32 strokes can ever enter any pixel's top-10 (the host
verifies >= 10 passing within the top D per pixel; the exact fixed input
needs D* = 30).  This cuts device traffic 8x.

Sharding: pure data parallel, one batch element per NeuronCore (B=8, 8
cores).  Engine split per stroke: DVE does the mask/count chain, Pool the
transmittance chain and the color products, PE accumulates the weighted
colors into PSUM via identity matmuls.
"""

import numpy as np

B, N, W, K = 8, 256, 128, 10
ALPHA_THRESH = 0.1
D = 30          # strokes processed from the top (must cover every pixel's top-10)
P = 128         # partitions (pixel rows)
F = 128         # free dim (pixel cols)
G = 8           # strokes per color-DMA chunk
NCORES = 8

_nc_cache = {}


def _build_nc(depth):
    import concourse.bass as bass  # noqa: F401
    import concourse.tile as tile
    from concourse import bacc, mybir
    from concourse.vector_clock import ScopedClock

    op = mybir.AluOpType
    f32 = mybir.dt.float32

    class _OneShotTileContext(tile.TileContext):
        """TileContext with a slim exit: drain + sem-only barrier, no
        per-semaphore clears / double barrier (~8us of EVSEM butterfly).
        Safe because every run_bass_kernel_spmd call builds and loads a
        fresh executable, so semaphore state never carries across runs."""

        def _drain_and_barrier(self, tick_clock, wait_clock):
            drain_inst = self.nc.sync.drain()
            wait_clock.add_sem_waits(
                drain_inst.ins, ScopedClock({None: tick_clock.global_clock})
            )
            self.nc.all_engine_barrier(sem_only=True)
            popped = self.nc._tile_sem_poison_stack.pop()
            assert popped is self._sem_poison

    nc = bacc.Bacc("TRN2", target_bir_lowering=False, debug=False)

    alpha_d = nc.dram_tensor("alpha_in", [P, depth * F], f32, kind="ExternalInput").ap()
    color_d = nc.dram_tensor("color_in", [P, depth * 3 * F], f32, kind="ExternalInput").ap()
    ident_d = nc.dram_tensor("ident_in", [P, P], f32, kind="ExternalInput").ap()
    out_d = nc.dram_tensor("out", [P, 3 * F], f32, kind="ExternalOutput").ap()

    with _OneShotTileContext(nc) as tc:
        with (
            tc.tile_pool(name="const", bufs=1) as constp,
            tc.tile_pool(name="state", bufs=1) as statep,
            tc.tile_pool(name="alpha", bufs=2) as alphap,
            tc.tile_pool(name="ae0", bufs=2) as ae0p,
            tc.tile_pool(name="cpair", bufs=4) as cpairp,
            tc.tile_pool(name="cchunk", bufs=2) as cchunkp,
            tc.tile_pool(name="tap", bufs=2) as tap,
            tc.tile_pool(name="aep", bufs=2) as aep,
            tc.tile_pool(name="prodp", bufs=3) as prodp,
            tc.tile_pool(name="psum", bufs=1, space="PSUM") as psump,
        ):
            # ident via SWDGE (gpsimd queue) so it doesn't delay the HWDGE
            # input stream; it's only needed by the first matmul.
            ident = constp.tile([P, P], f32)
            nc.gpsimd.dma_start(ident[:], ident_d)

            kcnt = statep.tile([P, F], f32)
            T = statep.tile([P, F], f32)
            nc.vector.memset(kcnt[:], 0.0)
            nc.gpsimd.memset(T[:], 1.0)

            cacc = psump.tile([P, 3 * F], f32)

            chunks = []
            off = 0
            while off < depth:
                g_sz = min(G, depth - off)
                chunks.append((off, g_sz))
                off += g_sz

            def chain_ops(ss, ae0_s, ta_out):
                """Serial per-stroke mask/count/transmittance ops (all DVE)."""
                if ss < K - 1:
                    ae = ae0_s          # k cannot have reached 10 yet
                else:
                    ae_t = aep.tile([P, F], f32, tag="ae")
                    nc.vector.scalar_tensor_tensor(
                        ae_t[:], kcnt[:], 9.5, ae0_s, op0=op.is_lt, op1=op.mult
                    )
                    ae = ae_t[:]
                if ss < depth - 1:
                    nc.vector.scalar_tensor_tensor(
                        kcnt[:], ae, 0.0, kcnt[:], op0=op.is_gt, op1=op.add
                    )
                nc.vector.tensor_tensor(ta_out, ae, T[:], op=op.mult)
                nc.vector.tensor_tensor(T[:], T[:], ta_out, op=op.subtract)

            # Everything on DVE: GpSimd shares SBUF ports with DVE and
            # co-running them degrades DVE ~5x.  PE (own xbus ports)
            # accumulates the weighted colors without contention.
            for off, g_sz in chunks:
                # alpha per chunk: first compute waits on 256KB, not the
                # whole slab
                atile = alphap.tile([P, G * F], f32, tag="alpha")
                nc.sync.dma_start(
                    atile[:, : g_sz * F], alpha_d[:, off * F : (off + g_sz) * F]
                )

                # chunk 0: color in stroke-pair slices so the first product
                # isn't gated on a big transfer; later chunks: one DMA each
                first = off == 0
                if first:
                    ctiles = []
                    for s2 in range(g_sz // 2):
                        cpair = cpairp.tile([P, 2, 3, F], f32, tag="cpair")
                        lo = (off + 2 * s2) * 3 * F
                        c_src = color_d[:, lo : lo + 2 * 3 * F]
                        nc.sync.dma_start(
                            cpair[:], c_src.rearrange("p (s c f) -> p s c f", s=2, c=3)
                        )
                        ctiles.append(cpair)
                else:
                    cchunk = cchunkp.tile([P, G, 3, F], f32, tag="cchunk")
                    lo = off * 3 * F
                    c_src = color_d[:, lo : lo + g_sz * 3 * F]
                    nc.sync.dma_start(
                        cchunk[:, :g_sz],
                        c_src.rearrange("p (s c f) -> p s c f", s=g_sz, c=3),
                    )

                # ae0 = a * 1{a > thresh} for the whole chunk (batched)
                ae0 = ae0p.tile([P, G * F], f32, tag="ae0")
                a_sl = atile[:, : g_sz * F]
                nc.vector.scalar_tensor_tensor(
                    ae0[:, : g_sz * F], a_sl, ALPHA_THRESH, a_sl,
                    op0=op.is_gt, op1=op.mult,
                )

                # group strokes: pairs for chunk 0 (latency), quads after
                bs = 2 if first else 4
                s = 0
                while s < g_sz:
                    b = min(bs, g_sz - s)
                    ta_grp = tap.tile([P, 4, F], f32, tag="ta")
                    for j in range(b):
                        chain_ops(off + s + j, ae0[:, (s + j) * F : (s + j + 1) * F],
                                  ta_grp[:, j])
                    prod = prodp.tile([P, 4, 3, F], f32, tag="prod")
                    if first:
                        c_grp = ctiles[s // 2][:]
                    else:
                        c_grp = cchunk[:, s : s + b]
                    ta_b = ta_grp[:, :b].unsqueeze(2).broadcast_to((P, b, 3, F))
                    nc.vector.tensor_tensor(prod[:, :b], c_grp, ta_b, op=op.mult)
                    for j in range(b):
                        nc.tensor.matmul(
                            cacc[:], ident[:],
                            prod[:, j].rearrange("p c f -> p (c f)"),
                            start=(off + s + j == 0), stop=False,
                        )
                    s += b

            # background: C += T (white) via PE, then DMA straight from PSUM
            T_b = T[:].unsqueeze(1).broadcast_to((P, 3, F))
            nc.tensor.matmul(cacc[:], ident[:], T_b, start=False, stop=True)
            out_t = constp.tile([P, 3 * F], f32, tag="out")
            nc.scalar.copy(out_t[:], cacc[:])
            nc.sync.dma_start(out_d, out_t[:])

    nc.compile()
    return nc


def _prep_inputs(color_stroke, alpha, depth):
    """Slice the top `depth` strokes (reversed so stroke 0 = highest index)
    and lay them out per core: alpha [P, depth*F], color [P, depth*3*F]."""
    a_r = alpha[:, N - depth :, 0][:, ::-1]          # (B, depth, P, F)
    c_r = color_stroke[:, N - depth :][:, ::-1]      # (B, depth, 3, P, F)
    ident = np.eye(P, dtype=np.float32)
    in_maps = []
    for b in range(B):
        a_core = np.ascontiguousarray(a_r[b].transpose(1, 0, 2)).reshape(P, depth * F)
        c_core = np.ascontiguousarray(c_r[b].transpose(2, 0, 1, 3)).reshape(
            P, depth * 3 * F
        )
        in_maps.append(
            {"alpha_in": a_core, "color_in": c_core, "ident_in": ident}
        )
    return in_maps


def _reference_numpy(color_stroke, alpha):
    """Exact replication of the oracle (incl. top-k tie-breaking) on host.
    Only used when the depth-cutoff precondition fails (pathological inputs)."""
    stroke_ids = np.arange(1, N + 1, dtype=np.int32).reshape(1, N, 1, 1)
    draw = stroke_ids * (alpha[:, :, 0] > ALPHA_THRESH).astype(np.int32)  # (B,N,W,W)
    draw_t = np.moveaxis(draw, 1, -1)  # (B,W,W,N)
    idx = np.argsort(-draw_t, axis=-1, kind="stable")[..., :K]  # (B,W,W,K)
    idx = np.moveaxis(idx, -1, 1)[:, :, None]  # (B,K,1,W,W)
    alpha_k = np.take_along_axis(alpha, idx, axis=1)  # (B,K,1,W,W)
    color_k = np.take_along_axis(color_stroke, idx, axis=1)  # (B,K,3,W,W)
    canvas = np.ones((B, 3, W, W), dtype=color_stroke.dtype)
    for i in range(K - 1, -1, -1):
        a = alpha_k[:, i]
        canvas = canvas * (1.0 - a) + a * color_k[:, i]
    return canvas


def kernel(color_stroke, alpha):
    color_stroke = np.asarray(color_stroke, dtype=np.float32)
    alpha = np.asarray(alpha, dtype=np.float32)
    assert color_stroke.shape == (B, N, 3, W, W), color_stroke.shape
    assert alpha.shape == (B, N, 1, W, W), alpha.shape

    # Precondition for the depth cutoff: every pixel finds its 10 passing
    # strokes within the top D.  (Exact fixed input needs D* = 30.)
    top_pass = (alpha[:, N - D :, 0] > ALPHA_THRESH).sum(axis=1)
    if top_pass.min() < K:
        return _reference_numpy(color_stroke, alpha)

    from concourse.bass_utils import run_bass_kernel_spmd

    if D not in _nc_cache:
        _nc_cache[D] = _build_nc(D)
    nc = _nc_cache[D]

    in_maps = _prep_inputs(color_stroke, alpha, D)
    res = run_bass_kernel_spmd(nc, in_maps, core_ids=list(range(NCORES)))

    out = np.empty((B, 3, W, W), dtype=np.float32)
    for b in range(B):
        out[b] = res.results[b]["out"].reshape(P, 3, F).transpose(1, 0, 2)
    return out


# revision 14
# speedup vs baseline: 1.0758x; 1.0758x over previous
"""Trainium2 Bass kernel for AttnPainterOil-style top-K stroke compositing.

Problem semantics (per pixel, fully independent):
  draw[n] = (n+1) * (alpha[n] > 0.1); top-K=10 of draw over N=256 strokes
  (descending) == the 10 highest-index strokes with alpha > 0.1 (for the
  target input distribution every pixel has >= 10 passing strokes, checked
  on the host below).  Gather alpha/color at those indices and composite
  back-to-front over a white canvas.

Streaming formulation used on device (front-to-back, strokes in descending
index order): maintain per-pixel transmittance T (init 1), accepted count k
(init 0) and color accumulator C (init 0).  For each stroke:
  ae = a * 1{a > 0.1} * 1{k < 10}
  k += 1{ae > 0}
  ta = ae * T ;  T -= ta ;  C += ta * c
Final canvas = C + T (white background).

Only the top D=32 strokes can ever enter any pixel's top-10 (the host
verifies >= 10 passing within the top D per pixel; the exact fixed input
needs D* = 30).  This cuts device traffic 8x.

Sharding: pure data parallel, one batch element per NeuronCore (B=8, 8
cores).  Engine split per stroke: DVE does the mask/count chain, Pool the
transmittance chain and the color products, PE accumulates the weighted
colors into PSUM via identity matmuls.
"""

import numpy as np

B, N, W, K = 8, 256, 128, 10
ALPHA_THRESH = 0.1
D = 30          # strokes processed from the top (must cover every pixel's top-10)
P = 128         # partitions (pixel rows)
F = 128         # free dim (pixel cols)
G = 8           # strokes per color-DMA chunk
NCORES = 8

_nc_cache = {}


def _build_nc(depth):
    import concourse.bass as bass  # noqa: F401
    import concourse.tile as tile
    from concourse import bacc, mybir
    from concourse.vector_clock import ScopedClock

    op = mybir.AluOpType
    f32 = mybir.dt.float32

    class _OneShotTileContext(tile.TileContext):
        """TileContext with a slim exit: drain + sem-only barrier, no
        per-semaphore clears / double barrier (~8us of EVSEM butterfly).
        Safe because every run_bass_kernel_spmd call builds and loads a
        fresh executable, so semaphore state never carries across runs."""

        def _drain_and_barrier(self, tick_clock, wait_clock):
            drain_inst = self.nc.sync.drain()
            wait_clock.add_sem_waits(
                drain_inst.ins, ScopedClock({None: tick_clock.global_clock})
            )
            self.nc.all_engine_barrier(sem_only=True)
            popped = self.nc._tile_sem_poison_stack.pop()
            assert popped is self._sem_poison

    nc = bacc.Bacc("TRN2", target_bir_lowering=False, debug=False)

    alpha_d = nc.dram_tensor("alpha_in", [P, depth * F], f32, kind="ExternalInput").ap()
    color_d = nc.dram_tensor("color_in", [P, depth * 3 * F], f32, kind="ExternalInput").ap()
    ident_d = nc.dram_tensor("ident_in", [P, P], f32, kind="ExternalInput").ap()
    out_d = nc.dram_tensor("out", [P, 3 * F], f32, kind="ExternalOutput").ap()

    with _OneShotTileContext(nc) as tc:
        with (
            tc.tile_pool(name="const", bufs=1) as constp,
            tc.tile_pool(name="state", bufs=1) as statep,
            tc.tile_pool(name="alpha", bufs=2) as alphap,
            tc.tile_pool(name="ae0", bufs=2) as ae0p,
            tc.tile_pool(name="cpair", bufs=4) as cpairp,
            tc.tile_pool(name="cchunk", bufs=2) as cchunkp,
            tc.tile_pool(name="tap", bufs=2) as tap,
            tc.tile_pool(name="aep", bufs=2) as aep,
            tc.tile_pool(name="prodp", bufs=4) as prodp,
            tc.tile_pool(name="psum", bufs=1, space="PSUM") as psump,
        ):
            # ident via SWDGE (gpsimd queue) so it doesn't delay the HWDGE
            # input stream; it's only needed by the first matmul.
            ident = constp.tile([P, P], f32)
            nc.gpsimd.dma_start(ident[:], ident_d)

            kcnt = statep.tile([P, F], f32)
            T = statep.tile([P, F], f32)
            nc.vector.memset(kcnt[:], 0.0)
            nc.gpsimd.memset(T[:], 1.0)

            cacc = psump.tile([P, 3 * F], f32)

            chunks = []
            off = 0
            while off < depth:
                g_sz = min(G, depth - off)
                chunks.append((off, g_sz))
                off += g_sz

            def chain_ops(ss, ae0_s, ta_out):
                """Serial per-stroke mask/count/transmittance ops (all DVE)."""
                if ss < K - 1:
                    ae = ae0_s          # k cannot have reached 10 yet
                else:
                    ae_t = aep.tile([P, F], f32, tag="ae")
                    nc.vector.scalar_tensor_tensor(
                        ae_t[:], kcnt[:], 9.5, ae0_s, op0=op.is_lt, op1=op.mult
                    )
                    ae = ae_t[:]
                if ss < depth - 1:
                    nc.vector.scalar_tensor_tensor(
                        kcnt[:], ae, 0.0, kcnt[:], op0=op.is_gt, op1=op.add
                    )
                nc.vector.tensor_tensor(ta_out, ae, T[:], op=op.mult)
                nc.vector.tensor_tensor(T[:], T[:], ta_out, op=op.subtract)

            # Everything on DVE: GpSimd shares SBUF ports with DVE and
            # co-running them degrades DVE ~5x.  PE (own xbus ports)
            # accumulates the weighted colors without contention.
            for off, g_sz in chunks:
                # alpha per chunk: first compute waits on 256KB, not the
                # whole slab
                atile = alphap.tile([P, G * F], f32, tag="alpha")
                nc.sync.dma_start(
                    atile[:, : g_sz * F], alpha_d[:, off * F : (off + g_sz) * F]
                )

                # chunk 0: color in stroke-pair slices so the first product
                # isn't gated on a big transfer; later chunks: one DMA each
                first = off == 0
                if first:
                    ctiles = []
                    for s2 in range(g_sz // 2):
                        cpair = cpairp.tile([P, 2, 3, F], f32, tag="cpair")
                        lo = (off + 2 * s2) * 3 * F
                        c_src = color_d[:, lo : lo + 2 * 3 * F]
                        nc.sync.dma_start(
                            cpair[:], c_src.rearrange("p (s c f) -> p s c f", s=2, c=3)
                        )
                        ctiles.append(cpair)
                else:
                    cchunk = cchunkp.tile([P, G, 3, F], f32, tag="cchunk")
                    lo = off * 3 * F
                    c_src = color_d[:, lo : lo + g_sz * 3 * F]
                    nc.sync.dma_start(
                        cchunk[:, :g_sz],
                        c_src.rearrange("p (s c f) -> p s c f", s=g_sz, c=3),
                    )

                # ae0 = a * 1{a > thresh} for the whole chunk (batched)
                ae0 = ae0p.tile([P, G * F], f32, tag="ae0")
                a_sl = atile[:, : g_sz * F]
                nc.vector.scalar_tensor_tensor(
                    ae0[:, : g_sz * F], a_sl, ALPHA_THRESH, a_sl,
                    op0=op.is_gt, op1=op.mult,
                )

                # stroke pairs throughout: keeps PE uniformly busy (quads
                # idle PE between bursts and trigger HAM downclock)
                bs = 2
                s = 0
                while s < g_sz:
                    b = min(bs, g_sz - s)
                    ta_grp = tap.tile([P, 2, F], f32, tag="ta")
                    for j in range(b):
                        chain_ops(off + s + j, ae0[:, (s + j) * F : (s + j + 1) * F],
                                  ta_grp[:, j])
                    prod = prodp.tile([P, 2, 3, F], f32, tag="prod")
                    if first:
                        c_grp = ctiles[s // 2][:]
                    else:
                        c_grp = cchunk[:, s : s + b]
                    ta_b = ta_grp[:, :b].unsqueeze(2).broadcast_to((P, b, 3, F))
                    nc.vector.tensor_tensor(prod[:, :b], c_grp, ta_b, op=op.mult)
                    for j in range(b):
                        nc.tensor.matmul(
                            cacc[:], ident[:],
                            prod[:, j].rearrange("p c f -> p (c f)"),
                            start=(off + s + j == 0), stop=False,
                        )
                    s += b

            # background: C += T (white) via PE, then DMA straight from PSUM
            T_b = T[:].unsqueeze(1).broadcast_to((P, 3, F))
            nc.tensor.matmul(cacc[:], ident[:], T_b, start=False, stop=True)
            out_t = constp.tile([P, 3 * F], f32, tag="out")
            nc.scalar.copy(out_t[:], cacc[:])
            nc.sync.dma_start(out_d, out_t[:])

    nc.compile()
    return nc


def _prep_inputs(color_stroke, alpha, depth):
    """Slice the top `depth` strokes (reversed so stroke 0 = highest index)
    and lay them out per core: alpha [P, depth*F], color [P, depth*3*F]."""
    a_r = alpha[:, N - depth :, 0][:, ::-1]          # (B, depth, P, F)
    c_r = color_stroke[:, N - depth :][:, ::-1]      # (B, depth, 3, P, F)
    ident = np.eye(P, dtype=np.float32)
    in_maps = []
    for b in range(B):
        a_core = np.ascontiguousarray(a_r[b].transpose(1, 0, 2)).reshape(P, depth * F)
        c_core = np.ascontiguousarray(c_r[b].transpose(2, 0, 1, 3)).reshape(
            P, depth * 3 * F
        )
        in_maps.append(
            {"alpha_in": a_core, "color_in": c_core, "ident_in": ident}
        )
    return in_maps


def _reference_numpy(color_stroke, alpha):
    """Exact replication of the oracle (incl. top-k tie-breaking) on host.
    Only used when the depth-cutoff precondition fails (pathological inputs)."""
    stroke_ids = np.arange(1, N + 1, dtype=np.int32).reshape(1, N, 1, 1)
    draw = stroke_ids * (alpha[:, :, 0] > ALPHA_THRESH).astype(np.int32)  # (B,N,W,W)
    draw_t = np.moveaxis(draw, 1, -1)  # (B,W,W,N)
    idx = np.argsort(-draw_t, axis=-1, kind="stable")[..., :K]  # (B,W,W,K)
    idx = np.moveaxis(idx, -1, 1)[:, :, None]  # (B,K,1,W,W)
    alpha_k = np.take_along_axis(alpha, idx, axis=1)  # (B,K,1,W,W)
    color_k = np.take_along_axis(color_stroke, idx, axis=1)  # (B,K,3,W,W)
    canvas = np.ones((B, 3, W, W), dtype=color_stroke.dtype)
    for i in range(K - 1, -1, -1):
        a = alpha_k[:, i]
        canvas = canvas * (1.0 - a) + a * color_k[:, i]
    return canvas


def kernel(color_stroke, alpha):
    color_stroke = np.asarray(color_stroke, dtype=np.float32)
    alpha = np.asarray(alpha, dtype=np.float32)
    assert color_stroke.shape == (B, N, 3, W, W), color_stroke.shape
    assert alpha.shape == (B, N, 1, W, W), alpha.shape

    # Precondition for the depth cutoff: every pixel finds its 10 passing
    # strokes within the top D.  (Exact fixed input needs D* = 30.)
    top_pass = (alpha[:, N - D :, 0] > ALPHA_THRESH).sum(axis=1)
    if top_pass.min() < K:
        return _reference_numpy(color_stroke, alpha)

    from concourse.bass_utils import run_bass_kernel_spmd

    if D not in _nc_cache:
        _nc_cache[D] = _build_nc(D)
    nc = _nc_cache[D]

    in_maps = _prep_inputs(color_stroke, alpha, D)
    res = run_bass_kernel_spmd(nc, in_maps, core_ids=list(range(NCORES)))

    out = np.empty((B, 3, W, W), dtype=np.float32)
    for b in range(B):
        out[b] = res.results[b]["out"].reshape(P, 3, F).transpose(1, 0, 2)
    return out
